# revision 16
# baseline (speedup 1.0000x reference)
"""Trainium2 Bass kernel for nn_BILINEAR_56169582297414 (gnn message passing).

Reference computation (per prediction pair b):
    item_e = item_table[item_inputs[b]]                    # [D]
    mem_e  = user_table[member_ids[b, :]]                  # [M, D]
    scores[m] = mem_e[m] @ W_bil @ item_e + b_bil          # bilinear
    w = scores * member_mask[b]                            # mask padded members
    fu = sum_m w[m] * mem_e[m]                             # [D]
    ne = [fu * item_e, fu, item_e]                         # [3D]
    y = sigmoid(relu(ne @ W1.T + b1) @ W2.T + b2)          # [1]

Strategy: data-parallel over 8 NeuronCores, tables replicated. The serial
resource is SWDGE descriptor generation on the GPSIMD Q7 (~10ns/desc on one
queue, ~2.5ns/desc aggregate across the 4 SWDGE queues), so:
  - rows are sorted by true group length (desc) and striped across cores so
    all cores share one per-tile max-length profile; each tile fetches only
    maxL members;
  - member gathers use a bf16 4-packed user table (256B elems, faster drain
    than 512B and half the HBM bytes), split into 4 per-queue gathers so
    descriptor generation runs on 4 Q7 lanes concurrently;
  - padded member slots gather a zero row (sentinel index) which makes their
    contribution vanish, eliminating the member_mask input entirely;
  - tiles are batched into variable-size groups (more tiles when maxL is
    small) so per-group fixed costs (DVE/ACT dispatch, aux DMAs) stay
    amortized across a roughly constant gather volume;
  - the 1-of-4 sub-row select runs on DVE in bf16 (2x rate) with the base
    copy on the Scalar/ACT engine; bilinear + MLP head on TensorE with
    PSUM stages chunked 4 tiles at a time.
"""

import sys

sys.path.insert(0, "/opt/trn_rl_repo")

import ml_dtypes
import numpy as np

B = 262144
M = 16
NU = 100000
NI = 50000
D = 32
N_CORES = 8
BC = B // N_CORES
P = 128
NT = BC // P

_COMPILED = {}


def _make_groups(prof):
    """Partition tiles into groups of (t0, g, GL). prof is non-increasing,
    GL = prof[t0]. More tiles per group when GL is small so the per-group
    gather volume (g*GL) stays roughly constant."""
    nt = len(prof)
    groups = []
    t = 0
    while t < nt:
        GL = int(prof[t])
        g = 4 if GL >= 8 else (8 if GL >= 4 else 16)
        g = min(g, nt - t)
        assert g % 4 == 0
        groups.append((t, g, GL))
        t += g
    # Execute big-g (long dependency chain) groups first so their compute
    # tails overlap later gathers; the final groups then drain quickly.
    groups.sort(key=lambda x: -x[1])
    return groups


def build_kernel(bc, prof):
    """Per-core Bass program. Member gathers hit a bf16 4-packed user table
    [NU//4+1, 128] (zero sentinel row for padded slots); item gathers hit the
    fp32 2-packed item table [NI//2, 64]."""
    import concourse.bacc as bacc
    import concourse.tile as tile
    from concourse import mybir
    from concourse.library_config import mlp

    nt = bc // P
    prof = [int(max(1, min(M, x))) for x in prof]
    groups = _make_groups(prof)
    dt = mybir.dt

    # per-group flat col offsets for the merged aux tensors
    i16_cols = [g * gl * 8 + g * 8 for (_, g, gl) in groups]   # ids + iid
    u8_cols = [3 * g * gl + g for (_, g, gl) in groups]        # msel*3 + isel
    i16_off = np.concatenate([[0], np.cumsum(i16_cols)]).astype(int)
    u8_off = np.concatenate([[0], np.cumsum(u8_cols)]).astype(int)

    nc = bacc.Bacc("TRN2", target_bir_lowering=False, debug=False,
                   num_swdge_queues=4)

    aux16 = nc.dram_tensor("aux16", [P, int(i16_off[-1])], dt.int16,
                           kind="ExternalInput")
    aux8 = nc.dram_tensor("aux8", [P, int(u8_off[-1])], dt.uint8,
                          kind="ExternalInput")
    user4 = nc.dram_tensor("user4", [NU // 4 + 1, 4 * D], dt.bfloat16,
                           kind="ExternalInput")
    item2 = nc.dram_tensor("item2", [NI // 2, 2 * D], dt.float32,
                           kind="ExternalInput")
    w_bil_t = nc.dram_tensor("w_bil_t", [D, D], dt.float32, kind="ExternalInput")
    w1_t = nc.dram_tensor("w1_t", [3 * D, 8], dt.float32, kind="ExternalInput")
    w2_t = nc.dram_tensor("w2_t", [8, 1], dt.float32, kind="ExternalInput")
    b1 = nc.dram_tensor("b1", [8, 1], dt.float32, kind="ExternalInput")
    b2 = nc.dram_tensor("b2", [1, 1], dt.float32, kind="ExternalInput")
    bbil = nc.dram_tensor("bbil", [P, 1], dt.float32, kind="ExternalInput")
    ident = nc.dram_tensor("ident", [P, P], dt.float32, kind="ExternalInput")
    y_out = nc.dram_tensor("y", [nt, P], dt.float32, kind="ExternalOutput")

    with tile.TileContext(nc) as tc:
        with (
            tc.tile_pool(name="const", bufs=1) as cpool,
            tc.tile_pool(name="io", bufs=8) as iopool,
            tc.tile_pool(name="work", bufs=3) as wpool,
            tc.tile_pool(name="gath", bufs=6) as gpool,
            tc.tile_pool(name="prodp", bufs=2) as prpool,
            tc.tile_pool(name="psum", bufs=1, space="PSUM") as ppool,
            tc.tile_pool(name="psumv", bufs=2, space="PSUM") as ppoolv,
        ):
            with tc.tile_critical():
                nc.gpsimd.load_library(mlp)

            wt_sb = cpool.tile([D, D], dt.float32, tag="wt")
            nc.sync.dma_start(out=wt_sb[:], in_=w_bil_t[:])
            w1_sb = cpool.tile([3 * D, 8], dt.float32, tag="w1")
            nc.sync.dma_start(out=w1_sb[:], in_=w1_t[:])
            w2_sb = cpool.tile([8, 1], dt.float32, tag="w2")
            nc.sync.dma_start(out=w2_sb[:], in_=w2_t[:])
            b1_sb = cpool.tile([8, 1], dt.float32, tag="b1")
            nc.sync.dma_start(out=b1_sb[:], in_=b1[:])
            b2_sb = cpool.tile([1, 1], dt.float32, tag="b2")
            nc.sync.dma_start(out=b2_sb[:], in_=b2[:])
            bbil_sb = cpool.tile([P, 1], dt.float32, tag="bbil")
            nc.sync.dma_start(out=bbil_sb[:], in_=bbil[:])
            id_sb = cpool.tile([P, P], dt.float32, tag="ident")
            nc.sync.dma_start(out=id_sb[:], in_=ident[:])

            for gi, (t0, g, GL) in enumerate(groups):
                gGL = g * GL
                nq4 = g // 4  # subtiles per queue

                a16_sb = iopool.tile([P, i16_cols[gi]], dt.int16, tag="a16")
                nc.sync.dma_start(
                    out=a16_sb[:],
                    in_=aux16[:, int(i16_off[gi]) : int(i16_off[gi + 1])],
                )
                ids_sb = a16_sb[:, : gGL * 8]
                iid_sb = a16_sb[:, gGL * 8 :]
                a8_sb = iopool.tile([P, u8_cols[gi]], dt.uint8, tag="a8")
                nc.sync.dma_start(
                    out=a8_sb[:],
                    in_=aux8[:, int(u8_off[gi]) : int(u8_off[gi + 1])],
                )
                ms_sb = [a8_sb[:, q * gGL : (q + 1) * gGL] for q in range(3)]
                is_sb = a8_sb[:, 3 * gGL :]

                # Member gather (bf16 4-packed, 256B elems): 4 per-queue
                # gathers over subtile ranges for concurrent desc-gen.
                g4_sb = gpool.tile([P, gGL * 4 * D], dt.bfloat16, tag="g4")
                g4 = g4_sb[:].rearrange("p (c e) -> p c e", c=gGL)
                nj = nq4 * GL * 128
                for q in range(4):
                    nc.gpsimd.dma_gather(
                        out_ap=g4[:, q * nq4 * GL : (q + 1) * nq4 * GL, :],
                        in_ap=user4[:],
                        idxs_ap=ids_sb[:, q * nq4 * GL * 8 : (q + 1) * nq4 * GL * 8],
                        num_idxs=nj,
                        num_idxs_reg=nj,
                        elem_size=4 * D,
                        single_packet=False,
                        queue_num=q,
                    )

                # 1-of-4 sub-row select -> mem [P, (g,GL), D] bf16.
                # Base copy on ACT, predicated copies on DVE (bf16 2x rate).
                mem_sb = wpool.tile([P, gGL * D], dt.bfloat16, tag="mem")
                mem3 = mem_sb[:].rearrange("p (c d) -> p c d", c=gGL)
                nc.scalar.activation(
                    out=mem3, in_=g4[:, :, 0:D],
                    func=mybir.ActivationFunctionType.Copy,
                )
                for q in range(3):
                    nc.vector.copy_predicated(
                        out=mem3,
                        mask=ms_sb[q].unsqueeze(2).broadcast_to([P, gGL, D]),
                        data=g4[:, :, (q + 1) * D : (q + 2) * D],
                    )

                # Item gather (fp32 2-packed, 256B elems); split across the
                # queues for big groups to keep the per-queue load balanced
                g2_sb = gpool.tile([P, g * 2 * D], dt.float32, tag="g2")
                g2 = g2_sb[:].rearrange("p (c e) -> p c e", c=g)
                if g >= 8:
                    for q in range(4):
                        nc.gpsimd.dma_gather(
                            out_ap=g2[:, q * nq4 : (q + 1) * nq4, :],
                            in_ap=item2[:],
                            idxs_ap=iid_sb[:, q * nq4 * 8 : (q + 1) * nq4 * 8],
                            num_idxs=nq4 * 128,
                            num_idxs_reg=nq4 * 128,
                            elem_size=2 * D,
                            single_packet=False,
                            queue_num=q,
                        )
                else:
                    nc.gpsimd.dma_gather(
                        out_ap=g2,
                        in_ap=item2[:],
                        idxs_ap=iid_sb,
                        num_idxs=g * 128,
                        num_idxs_reg=g * 128,
                        elem_size=2 * D,
                        single_packet=False,
                        queue_num=gi % 4,
                    )

                ne_sb = wpool.tile([P, g * 3 * D], dt.float32, tag="ne")
                ne3 = ne_sb[:].rearrange("p (g c) -> p g c", g=g)
                nc.scalar.activation(
                    out=ne3[:, :, 2 * D : 3 * D], in_=g2[:, :, 0:D],
                    func=mybir.ActivationFunctionType.Copy,
                )
                nc.vector.copy_predicated(
                    out=ne3[:, :, 2 * D : 3 * D],
                    mask=is_sb.unsqueeze(2).broadcast_to([P, g, D]),
                    data=g2[:, :, D : 2 * D],
                )

                # itemT + v per 4-subtile chunk (PSUM budget)
                v_sb = wpool.tile([P, g * D], dt.bfloat16, tag="vb")
                for ck in range(g // 4):
                    j0 = ck * 4
                    itemT_ps = ppool.tile([D, 4 * P], dt.float32, tag="itemT",
                                          space="PSUM")
                    for j in range(4):
                        nc.tensor.transpose(
                            out=itemT_ps[:, j * P : (j + 1) * P],
                            in_=ne3[:, j0 + j, 2 * D : 3 * D],
                            identity=id_sb[:],
                        )
                    itemT_sb = wpool.tile([D, 4 * P], dt.float32, tag="itemT")
                    nc.scalar.activation(
                        out=itemT_sb[:],
                        in_=itemT_ps[:],
                        func=mybir.ActivationFunctionType.Copy,
                    )
                    v_ps = ppoolv.tile([P, 4 * D], dt.float32, tag="v",
                                       space="PSUM")
                    for j in range(4):
                        nc.tensor.matmul(
                            v_ps[:, j * D : (j + 1) * D],
                            lhsT=itemT_sb[:, j * P : (j + 1) * P],
                            rhs=wt_sb[:],
                            start=True,
                            stop=True,
                        )
                    # v in bf16 so score/fu products run at DVE 2x rate
                    nc.scalar.activation(
                        out=v_sb[:, j0 * D : (j0 + 4) * D], in_=v_ps[:],
                        func=mybir.ActivationFunctionType.Copy,
                    )

                mem4 = mem_sb[:].rearrange("p (g m d) -> p g m d", g=g, m=GL)
                v_b = (
                    v_sb[:]
                    .rearrange("p (g d) -> p g d", g=g)
                    .unsqueeze(2)
                    .broadcast_to([P, g, GL, D])
                )
                prod_sb = prpool.tile([P, gGL * D], dt.bfloat16, tag="prod")
                prod4 = prod_sb[:].rearrange("p (g m d) -> p g m d", g=g, m=GL)
                nc.vector.tensor_mul(out=prod4, in0=mem4, in1=v_b)

                scores_sb = wpool.tile([P, gGL], dt.float32, tag="scores")
                sc3 = scores_sb[:].rearrange("p (g m) -> p g m", g=g)
                nc.vector.reduce_sum(
                    out=sc3, in_=prod4, axis=mybir.AxisListType.X
                )

                # w = scores + b_bil (ACT, bf16 out); padded slots hit the
                # zero member row so their contribution vanishes without a mask
                w_sb = wpool.tile([P, gGL], dt.bfloat16, tag="w")
                w3 = w_sb[:].rearrange("p (g m) -> p g m", g=g)
                nc.scalar.activation(
                    out=w3, in_=sc3,
                    func=mybir.ActivationFunctionType.Identity,
                    bias=bbil_sb[:, :1],
                )

                w_b = w3.unsqueeze(3).broadcast_to([P, g, GL, D])
                nc.vector.tensor_mul(out=prod4, in0=mem4, in1=w_b)

                nc.vector.reduce_sum(
                    out=ne3[:, :, D : 2 * D],
                    in_=prod_sb[:]
                    .rearrange("p (g m d) -> p g d m", g=g, m=GL),
                    axis=mybir.AxisListType.X,
                )

                nc.vector.tensor_mul(
                    out=ne3[:, :, 0:D],
                    in0=ne3[:, :, D : 2 * D],
                    in1=ne3[:, :, 2 * D : 3 * D],
                )

                # MLP head per 4-subtile chunk
                for ck in range(g // 4):
                    j0 = ck * 4
                    neT_ps = ppool.tile([3 * D, 4 * P], dt.float32, tag="neT",
                                        space="PSUM")
                    for j in range(4):
                        nc.tensor.transpose(
                            out=neT_ps[:, j * P : (j + 1) * P],
                            in_=ne3[:, j0 + j, :],
                            identity=id_sb[:],
                        )
                    neT_sb = wpool.tile([3 * D, 4 * P], dt.float32, tag="neTs")
                    nc.scalar.activation(
                        out=neT_sb[:],
                        in_=neT_ps[:],
                        func=mybir.ActivationFunctionType.Copy,
                    )
                    hT_ps = ppool.tile([8, 4 * P], dt.float32, tag="hT",
                                       space="PSUM")
                    for j in range(4):
                        nc.tensor.matmul(
                            hT_ps[:, j * P : (j + 1) * P],
                            lhsT=w1_sb[:],
                            rhs=neT_sb[:, j * P : (j + 1) * P],
                            start=True,
                            stop=True,
                        )
                    hT_sb = wpool.tile([8, 4 * P], dt.float32, tag="hTs")
                    nc.scalar.activation(
                        out=hT_sb[:],
                        in_=hT_ps[:],
                        func=mybir.ActivationFunctionType.Relu,
                        bias=b1_sb[:, :1],
                    )
                    yT_ps = ppool.tile([1, 4 * P], dt.float32, tag="yT",
                                       space="PSUM")
                    for j in range(4):
                        nc.tensor.matmul(
                            yT_ps[:, j * P : (j + 1) * P],
                            lhsT=w2_sb[:],
                            rhs=hT_sb[:, j * P : (j + 1) * P],
                            start=True,
                            stop=True,
                        )
                    y_sb = iopool.tile([1, 4 * P], dt.float32, tag="y")
                    nc.scalar.activation(
                        out=y_sb[:],
                        in_=yT_ps[:],
                        func=mybir.ActivationFunctionType.Sigmoid,
                        bias=b2_sb[:1, :1],
                    )
                    nc.sync.dma_start(
                        out=y_out[t0 + j0 : t0 + j0 + 4, :], in_=y_sb[:]
                    )

    nc.compile()
    return nc


def _lengths_from_mask(mask_b):
    mm = np.asarray(mask_b, dtype=bool)
    pos = np.arange(1, M + 1, dtype=np.int32)
    return (mm * pos[None, :]).max(axis=1).astype(np.int32)


def prepare(item_inputs, member_ids, member_mask, n_cores=N_CORES):
    L = _lengths_from_mask(member_mask)
    order = np.argsort(-L, kind="stable")
    n = len(L)
    bc = n // n_cores
    nt = bc // P
    Ls = L[order]
    prof = [int(max(1, Ls[t * P * n_cores])) for t in range(nt)]
    return order, prof


def _wrap16(idv):
    """[n] int16 idx list -> [128, n/16] wrapped + replicated layout."""
    n = len(idv)
    w16 = idv.reshape(n // 16, 16).T
    return np.tile(w16, (8, 1))


def _make_in_maps(item_inputs, member_ids, member_mask, user_table, item_table,
                  W_bil, b_bil, W1, b1, W2, b2, order, prof):
    item_inputs = np.asarray(item_inputs).astype(np.int32).reshape(-1)
    member_ids = np.asarray(member_ids).astype(np.int32)
    lengths = _lengths_from_mask(member_mask)
    # padded slots -> sentinel id NU whose 4-pack row is all zeros
    ids_eff = np.where(
        np.arange(M)[None, :] < lengths[:, None], member_ids, NU
    ).astype(np.int32)
    user4 = np.zeros((NU // 4 + 1, 4 * D), dtype=ml_dtypes.bfloat16)
    user4[: NU // 4] = (
        np.asarray(user_table, dtype=np.float32)
        .astype(ml_dtypes.bfloat16)
        .reshape(NU // 4, 4 * D)
    )
    item2 = np.ascontiguousarray(
        np.asarray(item_table, dtype=np.float32).reshape(NI // 2, 2 * D)
    )
    w_bil_t = np.ascontiguousarray(np.asarray(W_bil, dtype=np.float32).T)
    w1_t = np.ascontiguousarray(np.asarray(W1, dtype=np.float32).T)
    w2_t = np.ascontiguousarray(np.asarray(W2, dtype=np.float32).T)
    b1_c = np.asarray(b1, dtype=np.float32).reshape(8, 1)
    b2_c = np.asarray(b2, dtype=np.float32).reshape(1, 1)
    bbil_c = np.full((P, 1), np.asarray(b_bil, dtype=np.float32).reshape(-1)[0],
                     dtype=np.float32)
    ident = np.eye(P, dtype=np.float32)

    groups = _make_groups([int(max(1, min(M, x))) for x in prof])

    in_maps = []
    for c in range(N_CORES):
        rows = order[c::N_CORES]
        mi = ids_eff[rows]                 # [bc, M] with sentinel padding
        ii = item_inputs[rows]             # [bc]
        a16_parts, a8_parts = [], []
        for (t0, g, GL) in groups:
            blk = mi[t0 * P : (t0 + g) * P, :GL]              # [g*P, GL]
            b4 = blk.reshape(g, P, GL)
            idv = np.transpose(b4, (0, 2, 1)).reshape(-1)     # (j,m,p) order
            ib = ii[t0 * P : (t0 + g) * P].reshape(g, P)
            iv = ib.reshape(-1)                               # (j,p) order
            a16_parts.append(_wrap16((idv >> 2).astype(np.int16)))
            a16_parts.append(_wrap16((iv >> 1).astype(np.int16)))
            sub = (np.transpose(b4, (0, 2, 1)) & 3)           # [g, GL, P]
            subm = np.transpose(sub, (2, 0, 1)).reshape(P, g * GL)  # [p,(j,m)]
            for q in (1, 2, 3):
                a8_parts.append((subm == q).astype(np.uint8))
            a8_parts.append(((ib & 1).T).astype(np.uint8))    # [P, g]
        in_maps.append({
            "aux16": np.concatenate(a16_parts, axis=1),
            "aux8": np.concatenate(a8_parts, axis=1),
            "user4": user4,
            "item2": item2,
            "w_bil_t": w_bil_t,
            "w1_t": w1_t,
            "w2_t": w2_t,
            "b1": b1_c,
            "b2": b2_c,
            "bbil": bbil_c,
            "ident": ident,
        })
    return in_maps


def _get_compiled(prof):
    key = tuple(prof)
    if key not in _COMPILED:
        _COMPILED[key] = build_kernel(BC, list(prof))
    return _COMPILED[key]


def run_on_hw(nc, in_maps, trace=False):
    from concourse import bass_utils

    return bass_utils.run_bass_kernel_spmd(
        nc, in_maps, core_ids=list(range(N_CORES)), trace=trace
    )


def kernel(item_inputs, member_ids, member_mask, user_table, item_table,
           W_bil, b_bil, W1, b1, W2, b2):
    order, prof = prepare(item_inputs, member_ids, member_mask)
    nc = _get_compiled(prof)
    in_maps = _make_in_maps(item_inputs, member_ids, member_mask, user_table,
                            item_table, W_bil, b_bil, W1, b1, W2, b2, order, prof)
    res = run_on_hw(nc, in_maps, trace=False)
    y = np.empty(B, dtype=np.float32)
    for c in range(N_CORES):
        y[order[c::N_CORES]] = res.results[c]["y"].reshape(BC)
    return y.reshape(B, 1)
